# revision 1
# baseline (speedup 1.0000x reference)
"""GCN (2x GCNConv + linear + log_softmax) on 8 TRN2 NeuronCores (Bass/Tile).

Sharding: edge-cut. Core k owns dest nodes [k*nloc,(k+1)*nloc) and all edges into
them. Within a core, the 8 GPSIMD Q7 groups own contiguous equal dest-node ranges.

Gather: feature-sliced ap_gather. SBUF table [128, chunk] holds g^T (g = dinv*h,
16 feats) replicated per 16-partition group: partition 16g+f = feature f. Each Q7
group gathers its own per-edge source columns. Per-(node,chunk) slot runs padded
to x4 -> stride-4 DVE reduce -> chunk partials -> per-quarter ap_gather by dest
node (fixed S2Q slots/node) -> stride-S2Q reduce -> aggregate.

deg/dinv use the same two-stage path with messages = edge weights (no gather).
norm folded: out[n] = dinv[n]*sum(ew*(dinv*h)[src]) + b. AllGather exchanges
g^T shards between layers.
"""
import numpy as np

NCORES = 8
NGRP = 8
NCHUNK = 8
F = 16
G = 1792  # ap_gather call granularity (num_idxs per call)


# ---------------------------------------------------------------- host prep
def _prep_core(src, dst, w, node_lo, nloc, n_nodes, NI=None, S2Q=None):
    """Build slot/idx structures for one core. Returns dict (or just sizes)."""
    chunk_sz = n_nodes // NCHUNK
    npg = -(-nloc // NGRP)
    npg = ((npg + 15) // 16) * 16
    bounds = [min(nloc, g * npg) for g in range(NGRP + 1)]

    dl = dst - node_lo
    grp = np.minimum(dl // npg, NGRP - 1)
    q = src // chunk_sz
    sl = src % chunk_sz

    # sort edges by (grp, q, dest)
    order = np.lexsort((dl, q, grp))
    e_g, e_q, e_d, e_s, e_w = grp[order], q[order], dl[order], sl[order], w[order]

    key = (e_g * NCHUNK + e_q) * nloc + e_d
    uk, first, cnt = np.unique(key, return_index=True, return_counts=True)
    pad = ((cnt + 3) // 4) * 4
    gq = uk // nloc
    # run starts: grouped cumsum of pad within each gq segment
    cum = np.cumsum(pad) - pad
    seg_first = np.r_[True, gq[1:] != gq[:-1]]
    seg_base = cum[seg_first][np.cumsum(seg_first) - 1]
    run_start = cum - seg_base
    slots_gq = np.zeros(NGRP * NCHUNK, dtype=np.int64)
    np.add.at(slots_gq, gq, pad)
    NI_req = ((int(slots_gq.max()) + G - 1) // G) * G

    # stage2: partials per (node, quarter); CP = NI/4 per chunk
    nq_parts = pad // 4
    uk_node = uk % nloc
    uk_q = (uk // nloc) % NCHUNK
    s2cnt = np.zeros((4, nloc), dtype=np.int64)
    np.add.at(s2cnt, (uk_q // 2, uk_node), nq_parts)
    S2Q_req = int(s2cnt.max())
    if NI is None:
        return NI_req, S2Q_req

    CP = NI // 4
    assert 1 + 2 * CP <= 32767
    idx_slots = np.zeros((NCHUNK, NGRP, NI), dtype=np.int16)
    ew_slots = np.zeros((NCHUNK, NGRP, NI), dtype=np.float32)
    eidx = np.searchsorted(uk, key)  # uk row of each (sorted) edge
    pos = np.arange(len(order)) - first[eidx]
    slot = run_start[eidx] + pos
    idx_slots[e_q, e_g, slot] = e_s.astype(np.int16)
    ew_slots[e_q, e_g, slot] = e_w

    # stage2 gather indices: per quarter, per node: S2Q slots -> partial positions
    # partials buffer per quarter: [0]=zero, then chunk (2*qtr+j) partial c at
    # 1 + j*CP + (run_start//4 + c)
    NSq = npg * S2Q
    NS_pad = ((NSq + G - 1) // G) * G
    s2_idx = np.zeros((4, NGRP, NS_pad), dtype=np.int16)
    ordr = np.argsort((uk_q // 2) * nloc + uk_node, kind="stable")
    o_qtr = uk_q[ordr] // 2
    o_j = uk_q[ordr] % 2
    o_node = uk_node[ordr]
    o_g = gq[ordr] // NCHUNK
    o_nl = o_node - np.asarray(bounds)[o_g]
    o_k = nq_parts[ordr]
    o_base = 1 + o_j * CP + run_start[ordr] // 4
    gkey = o_qtr * nloc + o_node
    gfirst = np.r_[True, gkey[1:] != gkey[:-1]]
    csum = np.cumsum(o_k) - o_k
    f0 = csum - csum[gfirst][np.cumsum(gfirst) - 1]
    tot = int(o_k.sum())
    rep = np.repeat(np.arange(len(ordr)), o_k)
    ranges = np.arange(tot) - np.repeat(csum, o_k)
    flat_pos = (o_qtr[rep] * NGRP + o_g[rep]) * NS_pad \
        + o_nl[rep] * S2Q + f0[rep] + ranges
    s2_idx.reshape(-1)[flat_pos] = (o_base[rep] + ranges).astype(np.int16)

    def wrap(a):  # [NGRP, M] -> [128, M//16] int16 (idx j -> partition 16g+j%16)
        ng, m = a.shape
        out = np.zeros((128, m // 16), dtype=a.dtype)
        j = np.arange(m)
        for g in range(ng):
            out[16 * g + (j % 16), j // 16] = a[g]
        return out

    idx_w = np.stack([wrap(idx_slots[qq]) for qq in range(NCHUNK)])
    s2_w = np.stack([wrap(s2_idx[h]) for h in range(4)])
    return dict(idx=idx_w, s2=s2_w, ew=ew_slots,
                NI=NI, CP=CP, S2Q=S2Q, NS_pad=NS_pad, npg=npg, bounds=bounds)



def _bcast_mid(ap2d, k):
    from concourse.bass import AP
    return AP(ap2d.tensor, ap2d.offset, [ap2d.ap[0], [0, k], ap2d.ap[1]])


def _bcast_lead(ap2d, k):
    from concourse.bass import AP
    return AP(ap2d.tensor, ap2d.offset, [[0, k], ap2d.ap[0], ap2d.ap[1]])


# ---------------------------------------------------------------- device
def _build_program(nloc, n_nodes, NI, CP, S2Q, NS_pad, npg, bounds,
                   in_dim, hid, classes):
    import concourse.bass as bass
    import concourse.bacc as bacc
    import concourse.tile as tile
    from concourse import mybir
    from concourse.masks import make_identity

    chunk_sz = n_nodes // NCHUNK
    dt = mybir.dt.float32
    nc = bacc.Bacc("TRN2", target_bir_lowering=False, debug=False,
                   num_devices=NCORES)
    z_in = nc.dram_tensor("z", [nloc, in_dim], dt, kind="ExternalInput").ap()
    W1_in = nc.dram_tensor("W1", [in_dim, hid], dt, kind="ExternalInput").ap()
    b1_in = nc.dram_tensor("b1", [1, hid], dt, kind="ExternalInput").ap()
    W2_in = nc.dram_tensor("W2", [hid, hid], dt, kind="ExternalInput").ap()
    b2_in = nc.dram_tensor("b2", [1, hid], dt, kind="ExternalInput").ap()
    Wc_in = nc.dram_tensor("Wc", [hid, classes], dt, kind="ExternalInput").ap()
    bc_in = nc.dram_tensor("bc", [1, classes], dt, kind="ExternalInput").ap()
    idx_in = nc.dram_tensor("idx", [NCHUNK, 128, NI // 16], mybir.dt.int16,
                            kind="ExternalInput").ap()
    s2_in = nc.dram_tensor("s2", [4, 128, NS_pad // 16], mybir.dt.int16,
                           kind="ExternalInput").ap()
    ew_in = nc.dram_tensor("ew", [NCHUNK, NGRP, NI], dt, kind="ExternalInput").ap()
    out_ext = nc.dram_tensor("out", [nloc, classes], dt, kind="ExternalOutput").ap()
    dbg_deg = nc.dram_tensor("dbg_deg", [128, npg], dt, kind="ExternalOutput").ap()
    dbg_gt = nc.dram_tensor("dbg_gt", [NCORES * F, nloc], dt, kind="ExternalOutput").ap()
    dbg_x1 = nc.dram_tensor("dbg_x1", [F, nloc], dt, kind="ExternalOutput").ap()
    dbg_agg = nc.dram_tensor("dbg_agg", [128, npg], dt, kind="ExternalOutput").ap()

    gt_sh = nc.dram_tensor("gt_sh", [F, nloc], dt)
    gt_all = nc.dram_tensor("gt_all", [NCORES * F, nloc], dt, addr_space="Shared")
    hT_d = nc.dram_tensor("hT_d", [F, nloc], dt)
    xT_d = nc.dram_tensor("xT_d", [F, nloc], dt)

    with tile.TileContext(nc) as tc:
        with (
            tc.tile_pool(name="persist", bufs=1) as pers,
            tc.tile_pool(name="tab", bufs=1) as tabp,
            tc.tile_pool(name="work", bufs=2) as work,
            tc.tile_pool(name="parts", bufs=1) as partsp,
            tc.tile_pool(name="psum", bufs=1, space="PSUM") as psp,
            tc.tile_pool(name="small", bufs=1) as smallp,
        ):
            ident = smallp.tile([128, 128], dt, tag="ident")
            make_identity(nc, ident[:])
            idx_t = pers.tile([128, NCHUNK * (NI // 16)], mybir.dt.int16, tag="idx")
            for q in range(NCHUNK):
                nc.sync.dma_start(
                    out=idx_t[:, q * (NI // 16):(q + 1) * (NI // 16)],
                    in_=idx_in[q, :, :])
            s2_t = pers.tile([128, 4 * (NS_pad // 16)], mybir.dt.int16, tag="s2")
            for h in range(4):
                nc.sync.dma_start(
                    out=s2_t[:, h * (NS_pad // 16):(h + 1) * (NS_pad // 16)],
                    in_=s2_in[h, :, :])

            agg = pers.tile([128, npg], dt, tag="agg")
            dinv = pers.tile([128, npg], dt, tag="dinv")

            def aggregate(with_gather, out_tile):
                nc.vector.memset(out_tile[:], 0.0)
                for qtr in range(4):
                    parts = partsp.tile([128, 1 + 2 * CP], dt, tag="parts")
                    nc.vector.memset(parts[:, 0:1], 0.0)
                    for j in range(2):
                        q = 2 * qtr + j
                        tab = None
                        if with_gather:
                            tab = tabp.tile([128, chunk_sz], dt, tag="tab")
                            for g in range(NGRP):
                                nc.sync.dma_start(
                                    out=tab[16 * g:16 * g + F, :],
                                    in_=gt_all[F * q:F * (q + 1), :])
                        for c0 in range(0, NI, G):
                            ew_r = work.tile([128, G], dt, tag="ewrep")
                            full = ew_r[:]
                            for f in range(F):
                                from concourse.bass import AP as _AP
                                dst = _AP(full.tensor,
                                          full.offset + f * full.ap[0][0],
                                          [[full.ap[0][0] * 16, NGRP],
                                           full.ap[1]])
                                nc.sync.dma_start(out=dst,
                                                  in_=ew_in[q, :, c0:c0 + G])
                            if with_gather:
                                msg = work.tile([128, G], dt, tag="msg")
                                nc.gpsimd.ap_gather(
                                    out_ap=msg[:].rearrange(
                                        "p (n d) -> p n d", n=G, d=1),
                                    in_ap=tab[:].rearrange(
                                        "p (n d) -> p n d", n=chunk_sz, d=1),
                                    idxs_ap=idx_t[:, q * (NI // 16) + c0 // 16:
                                                  q * (NI // 16) + (c0 + G) // 16],
                                    channels=128, num_elems=chunk_sz, d=1,
                                    num_idxs=G)
                                nc.vector.tensor_tensor(
                                    out=msg[:], in0=msg[:], in1=ew_r[:],
                                    op=mybir.AluOpType.mult)
                            else:
                                msg = ew_r
                            nc.vector.tensor_reduce(
                                out=parts[:, 1 + j * CP + c0 // 4:
                                          1 + j * CP + (c0 + G) // 4]
                                    .rearrange("p (c o) -> p c o", c=G // 4, o=1),
                                in_=msg[:].rearrange("p (c s) -> p c s",
                                                     c=G // 4, s=4),
                                axis=mybir.AxisListType.X, op=mybir.AluOpType.add)
                    # stage 2 for this quarter
                    for c0 in range(0, NS_pad, G):
                        s2g = work.tile([128, G], dt, tag="s2g")
                        nc.gpsimd.ap_gather(
                            out_ap=s2g[:].rearrange("p (n d) -> p n d", n=G, d=1),
                            in_ap=parts[:].rearrange("p (n d) -> p n d",
                                                     n=1 + 2 * CP, d=1),
                            idxs_ap=s2_t[:, qtr * (NS_pad // 16) + c0 // 16:
                                         qtr * (NS_pad // 16) + (c0 + G) // 16],
                            channels=128, num_elems=1 + 2 * CP, d=1, num_idxs=G)
                        n0 = c0 // S2Q
                        n1 = min(npg, (c0 + G) // S2Q)
                        if n1 <= n0:
                            continue
                        red = work.tile([128, G // S2Q], dt, tag="red")
                        nc.vector.tensor_reduce(
                            out=red[:, :n1 - n0].rearrange("p (n o) -> p n o",
                                                           n=n1 - n0, o=1),
                            in_=s2g[:, :(n1 - n0) * S2Q]
                                .rearrange("p (n s) -> p n s", n=n1 - n0, s=S2Q),
                            axis=mybir.AxisListType.X, op=mybir.AluOpType.add)
                        nc.vector.tensor_tensor(
                            out=out_tile[:, n0:n1], in0=out_tile[:, n0:n1],
                            in1=red[:, :n1 - n0], op=mybir.AluOpType.add)

            # ---- pass 0: deg -> dinv
            aggregate(False, agg)
            nc.sync.dma_start(out=dbg_deg[:, :], in_=agg[:])
            nc.scalar.sqrt(dinv[:], agg[:])
            nc.vector.reciprocal(out=dinv[:], in_=dinv[:])

            # ---- h1^T (own shard) -> DRAM
            w1_t = smallp.tile([in_dim, hid], dt, tag="w1")
            nc.sync.dma_start(out=w1_t[:], in_=W1_in[:, :])
            ntile = -(-nloc // 128)
            for t in range(ntile):
                r0, r1 = t * 128, min(nloc, t * 128 + 128)
                m = r1 - r0
                zt = work.tile([128, in_dim], dt, tag="zt")
                nc.sync.dma_start(out=zt[:m, :], in_=z_in[r0:r1, :])
                ztp = psp.tile([128, 128], dt, tag="ztp", space="PSUM")
                nc.tensor.transpose(out=ztp[:in_dim, :m], in_=zt[:m, :],
                                    identity=ident[:m, :m])
                zts = work.tile([in_dim, 128], dt, tag="zts")
                nc.vector.tensor_copy(out=zts[:, :m], in_=ztp[:in_dim, :m])
                hp = psp.tile([F, 128], dt, tag="hp", space="PSUM")
                nc.tensor.matmul(out=hp[:, :m], lhsT=w1_t[:], rhs=zts[:, :m],
                                 start=True, stop=True)
                hs = work.tile([F, 128], dt, tag="hs")
                nc.vector.tensor_copy(out=hs[:, :m], in_=hp[:, :m])
                nc.sync.dma_start(out=hT_d[:, r0:r1], in_=hs[:, :m])

            def make_gT():  # gt_sh = dinv * hT_d ; then AllGather -> gt_all
                ht = work.tile([128, npg], dt, tag="gtmp")
                for g in range(NGRP):
                    lo, hi = bounds[g], bounds[g + 1]
                    if hi <= lo:
                        continue
                    nc.sync.dma_start(out=ht[16 * g:16 * g + F, :hi - lo],
                                      in_=hT_d[:, lo:hi])
                nc.vector.tensor_tensor(out=ht[:], in0=ht[:], in1=dinv[:],
                                        op=mybir.AluOpType.mult)
                for g in range(NGRP):
                    lo, hi = bounds[g], bounds[g + 1]
                    if hi <= lo:
                        continue
                    nc.sync.dma_start(out=gt_sh[:, lo:hi],
                                      in_=ht[16 * g:16 * g + F, :hi - lo])
                nc.gpsimd.collective_compute(
                    "AllGather", mybir.AluOpType.bypass,
                    replica_groups=[list(range(NCORES))],
                    ins=[gt_sh[:, :]], outs=[gt_all[:, :]])

            def scale_bias(bias_ap, relu, dst_dram):
                # dst = (relu?)(dinv*agg + b) -> DRAM [F, nloc]
                bvec = smallp.tile([128, 1], dt, tag="bias")
                for g in range(NGRP):
                    nc.sync.dma_start(
                        out=bvec[16 * g:16 * g + F, :],
                        in_=bias_ap[0:1, :].rearrange("o f -> f o"))
                tmp = work.tile([128, npg], dt, tag="gtmp")
                nc.vector.tensor_tensor(out=tmp[:], in0=agg[:], in1=dinv[:],
                                        op=mybir.AluOpType.mult)
                if relu:
                    nc.scalar.activation(tmp[:], tmp[:],
                                         mybir.ActivationFunctionType.Relu,
                                         bias=bvec[:, 0:1], scale=1.0)
                else:
                    nc.vector.tensor_tensor(
                        out=tmp[:], in0=tmp[:],
                        in1=bvec[:, 0:1].to_broadcast([128, npg]),
                        op=mybir.AluOpType.add)
                for g in range(NGRP):
                    lo, hi = bounds[g], bounds[g + 1]
                    if hi <= lo:
                        continue
                    nc.sync.dma_start(out=dst_dram[:, lo:hi],
                                      in_=tmp[16 * g:16 * g + F, :hi - lo])

            # ---- layer 1
            make_gT()
            nc.sync.dma_start(out=dbg_gt[:, :], in_=gt_all[:, :])
            aggregate(True, agg)
            nc.sync.dma_start(out=dbg_agg[:, :], in_=agg[:])
            scale_bias(b1_in, True, xT_d)
            nc.sync.dma_start(out=dbg_x1[:, :], in_=xT_d[:, :])

            # ---- h2^T = W2^T @ x1^T -> hT_d
            w2_t = smallp.tile([hid, hid], dt, tag="w2")
            nc.sync.dma_start(out=w2_t[:], in_=W2_in[:, :])
            for c0 in range(0, nloc, 512):
                c1 = min(nloc, c0 + 512)
                xt = work.tile([F, 512], dt, tag="xt")
                nc.sync.dma_start(out=xt[:, :c1 - c0], in_=xT_d[:, c0:c1])
                hp2 = psp.tile([F, 512], dt, tag="hp2", space="PSUM")
                nc.tensor.matmul(out=hp2[:, :c1 - c0], lhsT=w2_t[:],
                                 rhs=xt[:, :c1 - c0], start=True, stop=True)
                hs2 = work.tile([F, 512], dt, tag="hs2")
                nc.vector.tensor_copy(out=hs2[:, :c1 - c0], in_=hp2[:, :c1 - c0])
                nc.sync.dma_start(out=hT_d[:, c0:c1], in_=hs2[:, :c1 - c0])

            # ---- layer 2
            make_gT()
            aggregate(True, agg)
            scale_bias(b2_in, False, xT_d)

            # ---- classifier + log_softmax
            wc_t = smallp.tile([hid, classes], dt, tag="wc")
            nc.sync.dma_start(out=wc_t[:], in_=Wc_in[:, :])
            bc_t = smallp.tile([classes, 1], dt, tag="bc")
            nc.sync.dma_start(out=bc_t[:],
                              in_=bc_in[0:1, :].rearrange("o c -> c o"))
            for t in range(ntile):
                r0, r1 = t * 128, min(nloc, t * 128 + 128)
                m = r1 - r0
                xt2 = work.tile([F, 128], dt, tag="xt2")
                nc.sync.dma_start(out=xt2[:, :m], in_=xT_d[:, r0:r1])
                lp = psp.tile([classes, 128], dt, tag="lp", space="PSUM")
                nc.tensor.matmul(out=lp[:, :m], lhsT=wc_t[:], rhs=xt2[:, :m],
                                 start=True, stop=True)
                lg = work.tile([classes, 128], dt, tag="lg")
                nc.vector.tensor_tensor(out=lg[:, :m], in0=lp[:, :m],
                                        in1=bc_t[:, 0:1].to_broadcast([classes, m]),
                                        op=mybir.AluOpType.add)
                ltp = psp.tile([128, classes], dt, tag="ltp", space="PSUM")
                nc.tensor.transpose(out=ltp[:m, :], in_=lg[:, :m],
                                    identity=ident[:classes, :classes])
                lt = work.tile([128, classes], dt, tag="lt")
                nc.vector.tensor_copy(out=lt[:m, :], in_=ltp[:m, :])
                mx = work.tile([128, 1], dt, tag="mx")
                nc.vector.tensor_reduce(out=mx[:m, :], in_=lt[:m, :],
                                        axis=mybir.AxisListType.X,
                                        op=mybir.AluOpType.max)
                sh = work.tile([128, classes], dt, tag="sh")
                nc.vector.tensor_tensor(out=sh[:m, :], in0=lt[:m, :],
                                        in1=mx[:m, 0:1].to_broadcast([m, classes]),
                                        op=mybir.AluOpType.subtract)
                ex = work.tile([128, classes], dt, tag="ex")
                nc.scalar.activation(ex[:m, :], sh[:m, :],
                                     mybir.ActivationFunctionType.Exp)
                sm = work.tile([128, 1], dt, tag="sm")
                nc.vector.tensor_reduce(out=sm[:m, :], in_=ex[:m, :],
                                        axis=mybir.AxisListType.X,
                                        op=mybir.AluOpType.add)
                ls = work.tile([128, 1], dt, tag="ls")
                nc.scalar.activation(ls[:m, :], sm[:m, :],
                                     mybir.ActivationFunctionType.Ln)
                res = work.tile([128, classes], dt, tag="res")
                nc.vector.tensor_tensor(out=res[:m, :], in0=sh[:m, :],
                                        in1=ls[:m, 0:1].to_broadcast([m, classes]),
                                        op=mybir.AluOpType.subtract)
                nc.sync.dma_start(out=out_ext[r0:r1, :], in_=res[:m, :])
    nc.compile()
    return nc


def kernel(z, edge_index, edge_attr, W1, b1, W2, b2, Wc, bc, _trace=False):
    from concourse.bass_utils import run_bass_kernel_spmd

    z = np.asarray(z, dtype=np.float32)
    ei = np.asarray(edge_index)
    ea = np.asarray(edge_attr, dtype=np.float32)
    W1 = np.asarray(W1, np.float32); b1 = np.asarray(b1, np.float32)
    W2 = np.asarray(W2, np.float32); b2 = np.asarray(b2, np.float32)
    Wc = np.asarray(Wc, np.float32); bc = np.asarray(bc, np.float32)
    n, in_dim = z.shape
    hid = W1.shape[1]
    classes = Wc.shape[1]

    loops = np.arange(n, dtype=np.int64)
    src = np.concatenate([ei[0].astype(np.int64), loops])
    dst = np.concatenate([ei[1].astype(np.int64), loops])
    w = np.concatenate([ea, np.ones(n, np.float32)])

    nloc = n // NCORES
    core = dst // nloc
    NI = S2Q = 0
    masks = [core == c for c in range(NCORES)]
    for c in range(NCORES):
        m = masks[c]
        ni, s2 = _prep_core(src[m], dst[m], w[m], c * nloc, nloc, n)
        NI, S2Q = max(NI, ni), max(S2Q, s2)
    while G % S2Q != 0:
        S2Q += 1
    per_core = []
    for c in range(NCORES):
        m = masks[c]
        per_core.append(_prep_core(src[m], dst[m], w[m], c * nloc, nloc, n,
                                   NI=NI, S2Q=S2Q))
    pc0 = per_core[0]
    nc = _build_program(nloc, n, NI, pc0["CP"], S2Q, pc0["NS_pad"], pc0["npg"],
                        pc0["bounds"], in_dim, hid, classes)
    in_maps = []
    for c in range(NCORES):
        pc = per_core[c]
        in_maps.append({
            "z": np.ascontiguousarray(z[c * nloc:(c + 1) * nloc]),
            "W1": W1, "b1": b1.reshape(1, hid), "W2": W2,
            "b2": b2.reshape(1, hid), "Wc": Wc, "bc": bc.reshape(1, classes),
            "idx": pc["idx"], "s2": pc["s2"], "ew": pc["ew"],
        })
    try:
        res = run_bass_kernel_spmd(nc, in_maps, list(range(NCORES)), trace=_trace)
    except ModuleNotFoundError:
        res = run_bass_kernel_spmd(nc, in_maps, list(range(NCORES)))
    out = np.concatenate([res.results[c]["out"] for c in range(NCORES)], axis=0)
    if _trace:
        return out, res
    return out



# revision 7
# speedup vs baseline: 1.6557x; 1.6557x over previous
"""GCN (2x GCNConv + linear + log_softmax) on 8 TRN2 NeuronCores (Bass/Tile).

Layout: core c owns dest nodes [c*nloc,(c+1)*nloc) and all edges into them.
Within a core, gather group g (one Q7 core, 16 partitions) owns edges whose
SOURCE lies in shard g — so the AllGather output [8*16, nloc] (partition
16g+f = feature f of shard g) is the gather table directly, loaded once per
layer.

Host folds the full symmetric norm dinv[src]*ew*dinv[dst] into per-slot
weights (no degree pass on device). Per (dest-node, group) runs are padded
to x4 (light, deg<=16) or x16 (heavy), stride-4/16 DVE reduce produces
partials; a stage-2 ap_gather aligns <=4 partials per node, stride-4 reduce
then a PE matmul with a selector sums across the 8 groups. Edge weights are
replicated across the 16 feature partitions by a tiny PE matmul (sel8) into
PSUM instead of 16 HBM DMAs.
"""
import numpy as np

NCORES = 8
NG = 8          # gather groups = source shards
NQ = 4          # dest-node quarters (bounds partials SBUF)
G = 1536        # slots per gather/mult/reduce chunk (3 PSUM banks fp32)
LIGHT_MAX = 16  # deg <= 16 -> light (stride 4); else heavy (stride 16)


# ---------------------------------------------------------------- host prep
def _seg_cumsum_excl(x2d):
    c = np.cumsum(x2d, axis=1)
    return c - x2d


def _wrap16(a, ncols):
    """[NG, M] -> [16*NG, M//16]; group g element i -> row 16g+(i%16), col i//16."""
    ng, m = a.shape
    out = np.zeros((16 * ng, ncols), dtype=a.dtype)
    j = np.arange(m)
    for g in range(ng):
        out[16 * g + (j % 16), j // 16] = a[g]
    return out


def _prep(src, dst, w, n):
    """Build all per-core device inputs. Returns (statics, list of dicts)."""
    nloc = n // NCORES
    nq = nloc // NQ
    core = dst // nloc
    grp = src // nloc
    sl = src % nloc
    dl = dst % nloc
    q = dl // nq
    nl = dl % nq

    nb = NCORES * NQ * NG * nq
    bucket = ((core * NQ + q) * NG + grp) * nq + nl
    cnt = np.bincount(bucket, minlength=nb)
    assert cnt.max() <= 64, f"node in-degree per (node,group) too high: {cnt.max()}"
    heavy = cnt > LIGHT_MAX
    stride = np.where(heavy, 16, 4)
    slots = ((cnt + stride - 1) // stride) * stride
    lightslots = np.where(heavy, 0, slots)
    heavyslots = np.where(heavy, slots, 0)

    rows = NCORES * NQ * NG
    ls_cs = _seg_cumsum_excl(lightslots.reshape(rows, nq)).reshape(-1)
    hs_cs = _seg_cumsum_excl(heavyslots.reshape(rows, nq)).reshape(-1)
    Lsum = lightslots.reshape(NCORES, NQ, NG, nq).sum(3)
    Hsum = heavyslots.reshape(NCORES, NQ, NG, nq).sum(3)
    r = 64
    Lq = ((Lsum.max(axis=(0, 2)) + r - 1) // r * r).astype(np.int64)
    Hq = (np.maximum((Hsum.max(axis=(0, 2)) + r - 1) // r * r, r)).astype(np.int64)
    Qbase = np.concatenate([[0], np.cumsum(Lq + Hq)])
    NI = int(Qbase[-1])

    b_q = (bucket // (NG * nq)) % NQ  # quarter of each bucket... computed per edge below
    # per-bucket quarter index
    qb_idx = (np.arange(nb) // (NG * nq)) % NQ
    start = np.where(
        heavy,
        Qbase[qb_idx] + Lq[qb_idx] + hs_cs,
        Qbase[qb_idx] + ls_cs,
    )

    order = np.argsort(bucket, kind="stable")
    b_sorted = bucket[order]
    first = np.r_[True, b_sorted[1:] != b_sorted[:-1]]
    run_id = np.cumsum(first) - 1
    first_pos = np.flatnonzero(first)
    pos = np.arange(len(order)) - first_pos[run_id]
    slot = start[b_sorted] + pos

    e_core = core[order]
    e_grp = grp[order]
    flat = (e_core * NG + e_grp) * NI + slot
    idx_all = np.zeros(NCORES * NG * NI, np.int16)
    w_all = np.zeros(NCORES * NG * NI, np.float32)
    idx_all[flat] = sl[order].astype(np.int16)
    w_all[flat] = w[order]
    idx_all = idx_all.reshape(NCORES, NG, NI)
    w_all = w_all.reshape(NCORES, NG, NI)

    # stage-2: <=4 partial positions per (c,q,g,node), 1-based (0 = zero slot)
    npart = (cnt + stride - 1) // stride
    assert npart.max() <= 4
    pstart = np.where(heavy, 1 + Lq[qb_idx] // 4 + hs_cs // 16, 1 + ls_cs // 4)
    ar4 = np.arange(4)
    s2 = np.where(ar4[None, :] < npart[:, None], pstart[:, None] + ar4[None, :], 0)
    # [NC, NQ, NG, nq, 4] -> per (c,g): [NQ, nq*4] padded to s2cols*16
    s2 = s2.reshape(NCORES, NQ, NG, nq * 4).astype(np.int16)
    s2cols = (nq * 4 + 15) // 16
    s2pad = np.zeros((NCORES, NQ, NG, s2cols * 16), np.int16)
    s2pad[:, :, :, : nq * 4] = s2

    Pq = (Lq // 4 + Hq // 16).astype(np.int64)
    Pmax = int(Pq.max())
    assert 1 + Pmax <= 32767

    statics = dict(nloc=nloc, nq=nq, Lq=[int(x) for x in Lq],
                   Hq=[int(x) for x in Hq], Qbase=[int(x) for x in Qbase],
                   NI=NI, s2cols=int(s2cols), Pmax=Pmax)
    per_core = []
    for c in range(NCORES):
        idx_w = _wrap16(idx_all[c], NI // 16)           # [128, NI//16] i16
        s2_w = np.concatenate(
            [_wrap16(s2pad[c, qq], s2cols) for qq in range(NQ)], axis=1
        )                                               # [128, NQ*s2cols] i16
        per_core.append(dict(idx=np.ascontiguousarray(idx_w),
                             s2=np.ascontiguousarray(s2_w),
                             nrm=np.ascontiguousarray(w_all[c])))
    return statics, per_core


# ---------------------------------------------------------------- device
def _build_program(st, in_dim, hid, classes):
    import concourse.bass as bass
    import concourse.bacc as bacc
    import concourse.tile as tile
    from concourse import mybir
    from concourse.masks import make_identity

    nloc, nq = st["nloc"], st["nq"]
    Lq, Hq, Qbase, NI = st["Lq"], st["Hq"], st["Qbase"], st["NI"]
    s2cols, Pmax = st["s2cols"], st["Pmax"]
    S2T = s2cols * 16
    F = hid
    dt = mybir.dt.float32
    import os
    bf = mybir.dt.float32 if os.environ.get("GCN_FP32X") else mybir.dt.bfloat16
    AF = mybir.ActivationFunctionType
    nc = bacc.Bacc("TRN2", target_bir_lowering=False, debug=False,
                   num_devices=NCORES)

    zt_in = nc.dram_tensor("zt", [in_dim, nloc], dt, kind="ExternalInput").ap()
    W1_in = nc.dram_tensor("W1", [in_dim, F], dt, kind="ExternalInput").ap()
    b1_in = nc.dram_tensor("b1", [F, 1], dt, kind="ExternalInput").ap()
    W2_in = nc.dram_tensor("W2", [F, F], bf, kind="ExternalInput").ap()
    b2_in = nc.dram_tensor("b2", [F, 1], dt, kind="ExternalInput").ap()
    Wc_in = nc.dram_tensor("Wc", [F, classes], bf, kind="ExternalInput").ap()
    bc_in = nc.dram_tensor("bc", [classes, 1], dt, kind="ExternalInput").ap()
    sel8_in = nc.dram_tensor("sel8", [NG, 128], dt, kind="ExternalInput").ap()
    sel16_in = nc.dram_tensor("sel16", [128, F], dt, kind="ExternalInput").ap()
    onc_in = nc.dram_tensor("onc", [classes, 1], dt, kind="ExternalInput").ap()
    onr_in = nc.dram_tensor("onr", [1, classes], dt, kind="ExternalInput").ap()
    idx_in = nc.dram_tensor("idx", [128, NI // 16], mybir.dt.int16,
                            kind="ExternalInput").ap()
    s2_in = nc.dram_tensor("s2", [128, NQ * s2cols], mybir.dt.int16,
                           kind="ExternalInput").ap()
    nrm_in = nc.dram_tensor("nrm", [NG, NI], dt, kind="ExternalInput").ap()
    out_ext = nc.dram_tensor("out", [nloc, classes], dt, kind="ExternalOutput").ap()

    h_sh = nc.dram_tensor("h_sh", [F, nloc], dt)
    gt_all = nc.dram_tensor("gt_all", [NCORES * F, nloc], dt, addr_space="Shared")

    # per-quarter chunk lists: (c0_abs, size, stride, partial_base)
    def chunks_of(q):
        ch = []
        for c0 in range(0, Lq[q], G):
            sz = min(G, Lq[q] - c0)
            ch.append((Qbase[q] + c0, sz, 4, 1 + c0 // 4))
        hb = 1 + Lq[q] // 4
        for c0 in range(0, Hq[q], G):
            sz = min(G, Hq[q] - c0)
            ch.append((Qbase[q] + Lq[q] + c0, sz, 16, hb + c0 // 16))
        return ch

    with tile.TileContext(nc) as tc:
        with (
            tc.tile_pool(name="const", bufs=1) as constp,
            tc.tile_pool(name="table", bufs=1) as tablep,
            tc.tile_pool(name="parts", bufs=2) as partsp,
            tc.tile_pool(name="xbuf", bufs=1) as xp_pool,
            tc.tile_pool(name="work", bufs=3) as work,
            tc.tile_pool(name="ewsb", bufs=2) as ewsbp,
            tc.tile_pool(name="idxs", bufs=3) as idxp,
            tc.tile_pool(name="small", bufs=1) as smallp,
            tc.tile_pool(name="psA", bufs=1, space="PSUM") as psA,
            tc.tile_pool(name="psB", bufs=4, space="PSUM") as psB,
        ):
            ident = constp.tile([128, 128], dt, tag="ident")
            make_identity(nc, ident[:])
            sel8 = constp.tile([NG, 128], dt, tag="sel8")
            nc.sync.dma_start(out=sel8[:], in_=sel8_in[:, :])
            sel16 = constp.tile([128, F], dt, tag="sel16")
            nc.sync.dma_start(out=sel16[:], in_=sel16_in[:, :])
            w1t = constp.tile([in_dim, F], dt, tag="w1")
            nc.sync.dma_start(out=w1t[:], in_=W1_in[:, :])
            w2t = constp.tile([F, F], bf, tag="w2")
            nc.sync.dma_start(out=w2t[:], in_=W2_in[:, :])
            wct = constp.tile([F, classes], bf, tag="wc")
            nc.sync.dma_start(out=wct[:], in_=Wc_in[:, :])
            b1t = constp.tile([F, 1], dt, tag="b1")
            nc.sync.dma_start(out=b1t[:], in_=b1_in[:, :])
            b2t = constp.tile([F, 1], dt, tag="b2")
            nc.sync.dma_start(out=b2t[:], in_=b2_in[:, :])
            bct = constp.tile([classes, 1], dt, tag="bc")
            nc.sync.dma_start(out=bct[:], in_=bc_in[:, :])
            onc = constp.tile([classes, 1], dt, tag="onc")
            nc.sync.dma_start(out=onc[:], in_=onc_in[:, :])
            onr = constp.tile([1, classes], dt, tag="onr")
            nc.sync.dma_start(out=onr[:], in_=onr_in[:, :])
            s2t = constp.tile([128, NQ * s2cols], mybir.dt.int16, tag="s2")
            nc.sync.dma_start(out=s2t[:], in_=s2_in[:, :])

            table = tablep.tile([128, nloc], dt, tag="table")
            x_t = xp_pool.tile([F, nloc], bf, tag="x")

            # ---- h1 = W1^T @ z^T -> h_sh
            for j in range(0, nloc, 512):
                m = min(512, nloc - j)
                ztile = work.tile([in_dim, 512], dt, tag="zt", bufs=2)
                nc.sync.dma_start(out=ztile[:, :m], in_=zt_in[:, j:j + m])
                hp = psB.tile([128, 512], dt, tag="mm", space="PSUM")
                nc.tensor.matmul(out=hp[:F, :m], lhsT=w1t[:], rhs=ztile[:, :m],
                                 start=True, stop=True)
                hs = work.tile([F, 512], dt, tag="hs", bufs=2)
                nc.scalar.copy(out=hs[:, :m], in_=hp[:F, :m])
                nc.sync.dma_start(out=h_sh[:, j:j + m], in_=hs[:, :m])

            def allgather():
                nc.gpsimd.collective_compute(
                    "AllGather", mybir.AluOpType.bypass,
                    replica_groups=[list(range(NCORES))],
                    ins=[h_sh[:, :]], outs=[gt_all[:, :]])

            def aggregate(bias_t, relu):
                # table <- gt_all
                qn = nloc // 4
                for j in range(4):
                    nc.sync.dma_start(out=table[:, j * qn:(j + 1) * qn],
                                      in_=gt_all[:, j * qn:(j + 1) * qn])
                for q in range(NQ):
                    parts = partsp.tile([128, 1 + Pmax], dt, tag="parts")
                    nc.vector.memset(parts[:, 0:1], 0.0)
                    for (c0, sz, stride, pbase) in chunks_of(q):
                        idxt = idxp.tile([128, G // 16], mybir.dt.int16, tag="idx")
                        nc.sync.dma_start(
                            out=idxt[:, : sz // 16],
                            in_=idx_in[:, c0 // 16:(c0 + sz) // 16])
                        ewsb = ewsbp.tile([NG, G], dt, tag="ew")
                        nc.sync.dma_start(out=ewsb[:, :sz],
                                          in_=nrm_in[:, c0:c0 + sz])
                        ewps = psA.tile([128, G], dt, tag="ewps", space="PSUM")
                        for k in range(0, sz, 512):
                            m = min(512, sz - k)
                            nc.tensor.matmul(out=ewps[:, k:k + m],
                                             lhsT=sel8[:], rhs=ewsb[:, k:k + m],
                                             start=True, stop=True)
                        msg = work.tile([128, G], dt, tag="msg", bufs=2)
                        nc.gpsimd.ap_gather(
                            out_ap=msg[:, :sz].rearrange("p (n d) -> p n d",
                                                         n=sz, d=1),
                            in_ap=table[:].rearrange("p (n d) -> p n d",
                                                     n=nloc, d=1),
                            idxs_ap=idxt[:, : sz // 16],
                            channels=128, num_elems=nloc, d=1, num_idxs=sz)
                        nc.vector.tensor_tensor(out=msg[:, :sz], in0=msg[:, :sz],
                                                in1=ewps[:, :sz],
                                                op=mybir.AluOpType.mult)
                        nc.vector.tensor_reduce(
                            out=parts[:, pbase:pbase + sz // stride]
                                .rearrange("p (c o) -> p c o", c=sz // stride, o=1),
                            in_=msg[:, :sz].rearrange("p (c s) -> p c s",
                                                      c=sz // stride, s=stride),
                            axis=mybir.AxisListType.X, op=mybir.AluOpType.add)
                    # stage 2 of quarter q
                    for d0 in range(0, S2T, G):
                        sz2 = min(G, S2T - d0)
                        s2g = work.tile([128, G], dt, tag="s2g", bufs=2)
                        nc.gpsimd.ap_gather(
                            out_ap=s2g[:, :sz2].rearrange("p (n d) -> p n d",
                                                          n=sz2, d=1),
                            in_ap=parts[:].rearrange("p (n d) -> p n d",
                                                     n=1 + Pmax, d=1),
                            idxs_ap=s2t[:, q * s2cols + d0 // 16:
                                        q * s2cols + (d0 + sz2) // 16],
                            channels=128, num_elems=1 + Pmax, d=1, num_idxs=sz2)
                        red = work.tile([128, G // 4], dt, tag="red", bufs=2)
                        nc.vector.tensor_reduce(
                            out=red[:, : sz2 // 4].rearrange("p (c o) -> p c o",
                                                             c=sz2 // 4, o=1),
                            in_=s2g[:, :sz2].rearrange("p (c s) -> p c s",
                                                       c=sz2 // 4, s=4),
                            axis=mybir.AxisListType.X, op=mybir.AluOpType.add)
                        n0 = d0 // 4
                        for k in range(0, sz2 // 4, 512):
                            m = min(512, sz2 // 4 - k)
                            m = min(m, nq - (n0 + k))
                            if m <= 0:
                                break
                            xps = psB.tile([128, 512], dt, tag="mm", space="PSUM")
                            nc.tensor.matmul(out=xps[:F, :m], lhsT=sel16[:],
                                             rhs=red[:, k:k + m],
                                             start=True, stop=True)
                            col = q * nq + n0 + k
                            nc.scalar.activation(
                                x_t[:, col:col + m], xps[:F, :m],
                                AF.Relu if relu else AF.Identity,
                                bias=bias_t[:, 0:1], scale=1.0)

            # ---- layer 1
            allgather()
            aggregate(b1t, True)
            # ---- h2 = W2^T @ x1 -> h_sh
            for j in range(0, nloc, 512):
                m = min(512, nloc - j)
                hp2 = psB.tile([128, 512], dt, tag="mm", space="PSUM")
                nc.tensor.matmul(out=hp2[:F, :m], lhsT=w2t[:], rhs=x_t[:, j:j + m],
                                 start=True, stop=True)
                hs2 = work.tile([F, 512], dt, tag="hs", bufs=2)
                nc.scalar.copy(out=hs2[:, :m], in_=hp2[:F, :m])
                nc.sync.dma_start(out=h_sh[:, j:j + m], in_=hs2[:, :m])
            # ---- layer 2
            allgather()
            aggregate(b2t, False)

            # ---- classifier + log_softmax (no max subtraction; |logit| < 2)
            for j in range(0, nloc, 512):
                m = min(512, nloc - j)
                lgp = psB.tile([128, 512], dt, tag="mm", space="PSUM")
                nc.tensor.matmul(out=lgp[:classes, :m], lhsT=wct[:],
                                 rhs=x_t[:, j:j + m], start=True, stop=True)
                lgb = work.tile([classes, 512], dt, tag="lgb", bufs=1)
                nc.scalar.activation(lgb[:, :m], lgp[:classes, :m], AF.Identity,
                                     bias=bct[:, 0:1], scale=1.0)
                ex = work.tile([classes, 512], dt, tag="ex", bufs=1)
                nc.scalar.activation(ex[:, :m], lgb[:, :m], AF.Exp)
                sp = psB.tile([128, 512], dt, tag="mm", space="PSUM")
                nc.tensor.matmul(out=sp[:1, :m], lhsT=onc[:], rhs=ex[:, :m],
                                 start=True, stop=True)
                ls = smallp.tile([1, 512], dt, tag="ls")
                nc.scalar.activation(ls[:, :m], sp[:1, :m], AF.Ln)
                lsb = psB.tile([128, 512], dt, tag="mm", space="PSUM")
                nc.tensor.matmul(out=lsb[:classes, :m], lhsT=onr[:], rhs=ls[:, :m],
                                 start=True, stop=True)
                res = work.tile([classes, 512], dt, tag="res", bufs=1)
                nc.vector.tensor_tensor(out=res[:, :m], in0=lgb[:, :m],
                                        in1=lsb[:classes, :m],
                                        op=mybir.AluOpType.subtract)
                for k in range(0, m, 128):
                    mm = min(128, m - k)
                    tp = psB.tile([128, 512], dt, tag="mm", space="PSUM")
                    nc.tensor.transpose(out=tp[:mm, :classes], in_=res[:, k:k + mm],
                                        identity=ident[:classes, :classes])
                    tp = tp[:, :classes]
                    lt = work.tile([128, classes], dt, tag="lt", bufs=2)
                    nc.vector.tensor_copy(out=lt[:mm, :], in_=tp[:mm, :])
                    nc.sync.dma_start(out=out_ext[j + k:j + k + mm, :],
                                      in_=lt[:mm, :])
    nc.compile()
    return nc


# ---------------------------------------------------------------- entry
def kernel(z, edge_index, edge_attr, W1, b1, W2, b2, Wc, bc, _trace=False):
    from concourse.bass_utils import run_bass_kernel_spmd

    z = np.asarray(z, dtype=np.float32)
    ei = np.asarray(edge_index)
    ea = np.asarray(edge_attr, dtype=np.float32)
    W1 = np.asarray(W1, np.float32); b1 = np.asarray(b1, np.float32)
    W2 = np.asarray(W2, np.float32); b2 = np.asarray(b2, np.float32)
    Wc = np.asarray(Wc, np.float32); bc = np.asarray(bc, np.float32)
    n, in_dim = z.shape
    hid = W1.shape[1]
    classes = Wc.shape[1]
    nloc = n // NCORES

    loops = np.arange(n, dtype=np.int64)
    src = np.concatenate([ei[0].astype(np.int64), loops])
    dst = np.concatenate([ei[1].astype(np.int64), loops])
    ew = np.concatenate([ea, np.ones(n, np.float32)])
    deg = np.zeros(n, np.float32)
    np.add.at(deg, dst, ew)
    dinv = 1.0 / np.sqrt(deg)
    w = (dinv[src] * ew * dinv[dst]).astype(np.float32)

    import os
    _fp32x = bool(os.environ.get("GCN_FP32X"))
    try:
        from ml_dtypes import bfloat16 as _bf16
    except ImportError:
        import jax.numpy as jnp
        _bf16 = jnp.bfloat16
    statics, per_core = _prep(src, dst, w, n)
    nc = _build_program(statics, in_dim, hid, classes)

    F = hid
    sel8 = np.zeros((NG, 128), np.float32)
    for g in range(NG):
        sel8[g, 16 * g:16 * g + F] = 1.0
    sel16 = np.zeros((128, F), np.float32)
    for g in range(NG):
        sel16[16 * g:16 * g + F, :] = np.eye(F, dtype=np.float32)

    in_maps = []
    for c in range(NCORES):
        pc = per_core[c]
        in_maps.append({
            "zt": np.ascontiguousarray(z[c * nloc:(c + 1) * nloc].T),
            "W1": W1, "b1": b1.reshape(F, 1),
            "W2": np.asarray(W2, dtype=(np.float32 if _fp32x else _bf16)),
            "b2": b2.reshape(F, 1),
            "Wc": np.asarray(Wc, dtype=(np.float32 if _fp32x else _bf16)), "bc": bc.reshape(classes, 1),
            "sel8": sel8, "sel16": sel16,
            "onc": np.ones((classes, 1), np.float32),
            "onr": np.ones((1, classes), np.float32),
            "idx": pc["idx"], "s2": pc["s2"], "nrm": pc["nrm"],
        })

    try:
        res = run_bass_kernel_spmd(nc, in_maps, list(range(NCORES)), trace=_trace)
    except ModuleNotFoundError:
        res = run_bass_kernel_spmd(nc, in_maps, list(range(NCORES)))
    out = np.concatenate([res.results[c]["out"] for c in range(NCORES)], axis=0)
    if _trace:
        return out, res
    return out
